# revision 66
# baseline (speedup 1.0000x reference)
"""Trainium2 Bass kernel for nn_KnotEntangle (B=8, K=32, S=256, L=8).

Mathematically exact collapse of the reference:

1. smearWindow = [l, u] with l == u  =>  xStep == 0  =>  smear[b,k,:] is
   constant in s  =>  sig[b,k,:] = S*sigma[b,k]*delta_{n0} with
   sigma[b,k] = sum_l gauss((1-l)*x[b,k]; knot params).
2. corr[b,i,j] = S*sigma_i*sigma_j, so mix = gauss(outer; ent params).
3. result_re = sum_i sigma_i * (SQ2S*hre_i + (K-1) - r_i), with
   hre_i = sum_{j!=i} mix[j,i]*qre_j*sigma_j, r_i = sum_{j!=i} mix[j,i],
   [qre,qim] = P[:,0,0] * sin(pol + pi/4), SQ2S = S*sqrt2.  Collapsed on
   device into ONE [33,8]x[33,32] matmul (mix matrix augmented with a
   host-ones row carrying the (K-1) constant; W8 columns carry
   SQ2S*q*sigma - 1) followed by ONE sigma-weighted accumulate.
4. out[b,s] = g[b,s] * result[b], g = attention gate (sum of K gaussians),
   computed in a [128, 64] layout (knot k, s-quarter q on partition 4k+q)
   on the otherwise-idle GPSIMD engine.

Device-schedule design: only Exp activations (single act-table load hidden
under the input-DMA latency); diag(mix) killed by a BIG addend before the
Exp; per-knot gate scalars replicated across partitions with one PE
matmul; two input DMAs on independent queues; output is [8,64] (re/im x
s-quarter rows), reassembled on host.

Sharding: data-parallel over batch B (8 cores, one b each); knot params
replicated — the spec's sharding_hint.
"""

import math

import numpy as np
from ml_dtypes import bfloat16

import concourse.bacc as bacc
import concourse.mybir as mybir
import concourse.tile as tile
from concourse import bass_utils

B, K, S, L = 8, 32, 256, 8
F32 = mybir.dt.float32
BF16 = mybir.dt.bfloat16
AF = mybir.ActivationFunctionType
ALU = mybir.AluOpType
SQ2S = float(S * math.sqrt(2.0))
BIG = 1.0e9

# p32 column layout (f32, partitions 0..31 = knots)
C_X = 0
C_AW = 1
C_AB = 2
C_T34 = 3      # [1 - l*scope, 1 + u*scope]        (2 cols)
C_KM = 5       # kmean                              (8 cols)
C_EAH = 13     # exp(-2*khigh)                      (8 cols)
C_DVS = 21     # exp(-2*klow) - exp(-2*khigh)       (8 cols)
C_ONES32 = 29  # all-ones                           (32 cols)
C_EHM = 61     # exp(-2*ent_high) * S^2
C_DVM = 62     # exp(-2*ent_low) * S^2 - C_EHM
C_QQ8 = 64     # [SQ2S*qre, SQ2S*qim] * 4           (8 cols)
C_SPLIT = 61   # DMA1 = cols [0:61], DMA2 = cols [61:NC1]
C_BIGDM = 72   # BIG * I - ent_mean/S (bcast)       (32 cols)
C_R = 104      # R[k, p] = (p // 4 == k)            (128 cols)
C_NEG1 = 232   # -1.0
NC1 = 233

NC2 = 72       # rq (bf16, 128 partitions): ramp (64) + QQ8sel (8)

_NC_CACHE = {}


def _build_nc(one_minus_l: float) -> bacc.Bacc:
    nc = bacc.Bacc("TRN2", target_bir_lowering=False, debug=False)
    p32_d = nc.dram_tensor("p32", [K, NC1], F32, kind="ExternalInput")
    rq_d = nc.dram_tensor("rq", [128, NC2], BF16, kind="ExternalInput")
    out_d = nc.dram_tensor("out", [8, 64], F32, kind="ExternalOutput")

    with tile.TileContext(nc) as tc:
        with (
            tc.tile_pool(name="sb", bufs=1) as sb,
            tc.tile_pool(name="ps", bufs=8, space="PSUM") as ps,
        ):
            p32 = sb.tile([K, NC1], F32)
            rq = sb.tile([128, NC2], BF16)
            # Three input DMAs: sigma-critical columns first.
            nc.sync.dma_start(p32[:, 0:C_SPLIT], p32_d.ap()[:, 0:C_SPLIT])
            nc.scalar.dma_start(p32[:, C_SPLIT:NC1],
                                p32_d.ap()[:, C_SPLIT:NC1])
            nc.gpsimd.dma_start(rq[:], rq_d.ap()[:, :])

            x_c = p32[:, C_X:C_X + 1]
            aw_c = p32[:, C_AW:C_AW + 1]
            ab_c = p32[:, C_AB:C_AB + 1]
            t34 = p32[:, C_T34:C_T34 + 2]
            km = p32[:, C_KM:C_KM + 8]
            eahS = p32[:, C_EAH:C_EAH + 8]
            dvS = p32[:, C_DVS:C_DVS + 8]
            eHm_c = p32[:, C_EHM:C_EHM + 1]
            dvm_c = p32[:, C_DVM:C_DVM + 1]
            qq8 = p32[:, C_QQ8:C_QQ8 + 8]
            ones32 = p32[:, C_ONES32:C_ONES32 + 32]
            bigdm = p32[:, C_BIGDM:C_BIGDM + 32]
            Rm = p32[:, C_R:C_R + 128]
            neg1 = p32[:, C_NEG1:C_NEG1 + 1]
            ramp = rq[:, 0:64]
            qsel = rq[:, 64:72]

            # scratch tiles
            sc = sb.tile([K, 10], F32)       # 1 am | 2:4 aLH | 6:10 scal4
            nds = sb.tile([K, L], F32)
            mss = sb.tile([K, L], F32)
            d2s = sb.tile([K, L], F32)
            sels = sb.tile([K, L], F32)
            z2s = sb.tile([K, L], F32)
            esm = sb.tile([K, L], F32)
            sgv = sb.tile([K, 1], F32)       # sigma
            sgT_in = sb.tile([K, 32], F32)   # sigma bc to 32 cols
            sgTT = sb.tile([K, 32], F32)     # every row = sigma^T
            dMt = sb.tile([K, K], F32)
            mdMt = sb.tile([K, K], F32)
            d2Mt = sb.tile([K, K], F32)
            W8 = sb.tile([33, 8], BF16)      # [SQ2S*q*sg - (1,0)] ; row32=(K-1,0)
            jk8 = sb.tile([8, K], F32)       # stt junk out
            res8 = sb.tile([8, 1], F32)
            rcp = sb.tile([128, 4], F32)
            dGt = sb.tile([128, 64], BF16)
            mdGt = sb.tile([128, 64], BF16)
            mdGt2 = sb.tile([128, 64], BF16)
            d2Gt = sb.tile([128, 64], BF16)
            # combined Exp input/output: cols 0:64 gate z^2 (128p);
            # cols 64:96 mix z^2 (partitions 0:32; 32:128 zeroed -> exp = 1,
            # so eALL[0:33, 64:96] is the ones-augmented mix matrix)
            zALL = sb.tile([128, 96], BF16)
            eALL = sb.tile([128, 96], BF16)
            out8 = sb.tile([8, 64], F32)

            xsum = ps.tile([K, 1], F32, tag="ps")
            rep4 = ps.tile([128, 4], F32, tag="ps")
            s3T8 = ps.tile([8, K], F32, tag="ps")
            gP8 = ps.tile([8, 64], F32, tag="ps")

            sg_c = sgv[:, 0:1]
            scal4 = sc[:, 6:10]

            # Pool: constant rows / zero filler (no deps)
            nc.gpsimd.memset(zALL[32:33, 64:96], 0.0)
            nc.gpsimd.memset(W8[32:33, 0:8:2], float(K - 1))
            nc.gpsimd.memset(W8[32:33, 1:8:2], 0.0)

            # PE: broadcast sum(x) to all 32 partitions.
            nc.tensor.matmul(xsum[:], ones32, x_c)

            # ---- sigma chain (DVE spine) ----
            nc.vector.scalar_tensor_tensor(nds[:],
                                           x_c.broadcast_to([K, L]),
                                           -one_minus_l, km, ALU.mult,
                                           ALU.add)
            nc.vector.scalar_tensor_tensor(mss[:], nds[:], 0.0, dvS,
                                           ALU.is_ge, ALU.mult)
            nc.vector.tensor_mul(d2s[:], nds[:], nds[:])
            nc.vector.tensor_add(sels[:], mss[:], eahS)
            nc.vector.tensor_mul(z2s[:], d2s[:], sels[:])
            # gate smalls (DVE; fill the sigma-chain RAW-latency gaps)
            nc.vector.tensor_scalar(sc[:, 2:4], t34, xsum[:], 1.0 / K,
                                    ALU.mult, ALU.mult)
            nc.scalar.activation(sc[:, 1:2], x_c, AF.Identity, bias=ab_c,
                                 scale=aw_c)
            nc.vector.tensor_sub(sc[:, 6:7], sc[:, 3:4], sc[:, 2:3])  # diffc
            nc.vector.tensor_sub(sc[:, 7:8], sc[:, 2:3], sc[:, 1:2])  # aLm

            # Act: gate window exps first (ready earlier), then sigma
            nc.scalar.activation(sc[:, 8:10], sc[:, 2:4], AF.Exp, scale=-2.0)
            nc.scalar.activation(esm[:], z2s[:], AF.Exp, scale=-0.5)

            # DVE: dvg in place (fills the esm wait gap)
            nc.vector.tensor_sub(sc[:, 8:9], sc[:, 8:9], sc[:, 9:10])  # dvg

            # PE: replicate gate scalars to 128 partitions
            nc.tensor.matmul(rep4[:], Rm, scal4)

            # Act: PSUM->SBUF copy of the replicated gate scalars
            nc.scalar.activation(rcp[:], rep4[:], AF.Identity)

            # DVE sigma spine
            nc.vector.tensor_reduce(sg_c, esm[:], mybir.AxisListType.X,
                                    ALU.add)
            nc.vector.transpose(sgTT[:], sg_c.broadcast_to([K, 32]))

            # Act: W8 = qq8 * sigma - (1 on even cols)
            nc.scalar.activation(W8[0:K, 0:8:2], qq8[:, 0:8:2], AF.Identity,
                                 bias=neg1, scale=sg_c)
            nc.scalar.activation(W8[0:K, 1:8:2], qq8[:, 1:8:2], AF.Identity,
                                 scale=sg_c)

            # ---- mix mid chain (DVE); sgTT[j,i] = sigma_i ----
            # dM = sigma_j*sigma_i + (BIG*I - em_j)  (diag killed pre-Exp)
            nc.vector.scalar_tensor_tensor(dMt[:], sgTT[:], sg_c, bigdm,
                                           ALU.mult, ALU.add)
            nc.vector.tensor_scalar(mdMt[:], dMt[:], 0.0, dvm_c, ALU.is_le,
                                    ALU.mult)
            nc.vector.tensor_mul(d2Mt[:], dMt[:], dMt[:])
            nc.vector.scalar_tensor_tensor(zALL[0:K, 64:96], mdMt[:], eHm_c,
                                           d2Mt[:], ALU.add, ALU.mult)

            # Gate big chain: dG and dG^2 on the idle Act engine; the
            # mask and z^2 assembly stay on DVE
            nc.scalar.activation(dGt[:], ramp, AF.Identity, bias=rcp[:, 1:2],
                                 scale=rcp[:, 0:1])
            nc.vector.tensor_mul(d2Gt[:], dGt[:], dGt[:])
            nc.vector.tensor_scalar(mdGt[:], dGt[:], 0.0, rcp[:, 2:3],
                                    ALU.is_le, ALU.mult)
            nc.vector.scalar_tensor_tensor(zALL[:, 0:64], mdGt[:],
                                           rcp[:, 3:4], d2Gt[:], ALU.add,
                                           ALU.mult)

            # Act: gate exp (finishes first), then mix exp ([33,32]: the
            # zeroed row 32 becomes exp(0) = 1, the ones-augmentation row)
            nc.scalar.activation(eALL[:, 0:64], zALL[:, 0:64], AF.Exp,
                                 scale=-0.5)
            nc.scalar.activation(eALL[0:33, 64:96], zALL[0:33, 64:96],
                                 AF.Exp, scale=-0.5)

            # PE: one matmul for the whole pairwise tail; gate reduction
            nc.tensor.matmul(s3T8[:], W8[:], eALL[0:33, 64:96])
            nc.tensor.matmul(gP8[:], qsel, eALL[:, 0:64])

            # DVE: res8[r] = sum_i s3T8[r,i]*sigma_i ; then final scale
            nc.vector.scalar_tensor_tensor(jk8[:], s3T8[:], 0.0,
                                           sgTT[0:8, :], ALU.add, ALU.mult,
                                           accum_out=res8[:])
            nc.vector.tensor_scalar(out8[:], gP8[:], res8[:], None, ALU.mult)
            nc.sync.dma_start(out_d.ap()[:, :], out8[:])

    nc.compile()
    return nc


def _prep_in_maps(inputs):
    x = np.ascontiguousarray(inputs["x"], dtype=np.float32)
    sw = np.asarray(inputs["smearWindow"], dtype=np.float32)
    if not float(sw[0]) == float(sw[1]):
        raise NotImplementedError(
            "kernel specialized for smearWindow[0] == smearWindow[1] "
            "(xStep == 0); got %r" % (sw,)
        )
    l = float(sw[0])
    u = float(sw[1])
    scope = np.asarray(inputs["attn_scope"], np.float64)
    kl = np.asarray(inputs["klow"], np.float64)
    kh = np.asarray(inputs["khigh"], np.float64)
    el = np.asarray(inputs["ent_low"], np.float64)
    eh = np.asarray(inputs["ent_high"], np.float64)
    pol = np.asarray(inputs["pol"], np.float64)

    base = np.zeros((K, NC1), dtype=np.float32)
    base[:, C_AW] = inputs["attn_w"]
    base[:, C_AB] = inputs["attn_b"]
    base[:, C_T34] = 1.0 - l * scope
    base[:, C_T34 + 1] = 1.0 + u * scope
    base[:, C_KM:C_KM + 8] = inputs["kmean"]
    eahS = np.exp(-2.0 * kh)
    base[:, C_EAH:C_EAH + 8] = eahS
    base[:, C_DVS:C_DVS + 8] = np.exp(-2.0 * kl) - eahS
    eHm = np.exp(-2.0 * eh) * (S * S)
    base[:, C_EHM] = eHm
    base[:, C_DVM] = np.exp(-2.0 * el) * (S * S) - eHm
    s2p = np.sin(pol + math.pi / 4.0) * SQ2S
    qre = np.asarray(inputs["pol_re"][:, 0, 0], np.float64) * s2p
    qim = np.asarray(inputs["pol_im"][:, 0, 0], np.float64) * s2p
    base[:, C_QQ8 + 0:C_QQ8 + 8:2] = qre[:, None]
    base[:, C_QQ8 + 1:C_QQ8 + 8:2] = qim[:, None]
    base[:, C_ONES32:C_ONES32 + 32] = 1.0
    base[:, C_NEG1] = -1.0
    base[:, C_BIGDM:C_BIGDM + 32] = (
        BIG * np.eye(K) - (np.asarray(inputs["ent_mean"], np.float64) / S)[:, None]
    ).astype(np.float32)
    pidx = np.arange(128)
    base[:, C_R:C_R + 128] = (pidx[None, :] // 4 ==
                              np.arange(K)[:, None]).astype(np.float32)

    rq = np.zeros((128, NC2), dtype=bfloat16)
    sp = np.arange(64)
    rq[:, 0:64] = (((pidx[:, None] % 4) * 64 + sp[None, :] + 1.0) /
                   S).astype(bfloat16)
    qsel = np.zeros((128, 8), dtype=np.float32)
    for c in range(8):
        qsel[:, c] = (pidx % 4 == c // 2)
    rq[:, 64:72] = qsel.astype(bfloat16)

    in_maps = []
    for b in range(B):
        p32 = base.copy()
        p32[:, C_X] = x[b]
        in_maps.append({"p32": p32, "rq": rq})
    return in_maps, 1.0 - l


LAST_RESULTS = None


def kernel(**inputs) -> np.ndarray:
    global LAST_RESULTS
    import os

    in_maps, one_minus_l = _prep_in_maps(inputs)
    ckey = ("nc", round(one_minus_l, 12))
    if ckey not in _NC_CACHE:
        _NC_CACHE[ckey] = _build_nc(one_minus_l)
    nc = _NC_CACHE[ckey]
    _NC_CACHE["nc"] = nc  # for test.py introspection
    trace = bool(int(os.environ.get("KNOT_TRACE", "0")))
    r = bass_utils.run_bass_kernel_spmd(
        nc, in_maps, core_ids=list(range(B)), trace=trace
    )
    LAST_RESULTS = r
    out = np.empty((B, S), dtype=np.complex64)
    for b in range(B):
        o = np.asarray(r.results[b]["out"], dtype=np.float32)  # [8, 64]
        out[b] = (o[0::2] + 1j * o[1::2]).reshape(S)
    return out
